# revision 1
# baseline (speedup 1.0000x reference)
"""ArcFace loss (m=0.5, s=40) on 8 TRN2 NeuronCores.

Full inputs -> batch-sharded across 8 cores (rows), each core computes the
loss contribution of its 256 rows fully locally, host sums 8 partial scalars.

Per-row math (margin only touches the label column):
    rowsum = sum_c exp(S * x[c])
    adj    = rowsum - exp(S * x_lbl) + exp(S * phi(x_lbl))
    loss   = log(adj) - S * phi(x_lbl)
S * x < 40, so exp never overflows f32 and no max-subtraction pass is needed.
log(adj) is computed as Ln(adj * 2^-40) + 40*ln2 to stay inside the ScalarE
Ln table's valid input range (+-2^64).
sqrt(1-x^2) is computed as exp(0.5*ln(1-x^2)) so every ACT op lives in the
single natural_log_exp table set (one table load total).
"""

import math

import numpy as np

import concourse.bacc as bacc
import concourse.mybir as mybir
import concourse.tile as tile
from concourse.bass_isa import ReduceOp
from concourse.bass_utils import run_bass_kernel_spmd

# Problem shape (hardcoded per harness contract).
N, C = 2048, 32768
NCORES = 8
R = N // NCORES  # rows per core = 256
P = 128  # SBUF partitions
RB = R // P  # row blocks per core = 2
F = 8192  # columns per DMA tile
T = C // F  # tiles per row block = 4

# ArcFace constants (m=0.5, s=40).
M_MARGIN = 0.5
S = 40.0
SIN_M = math.sin(M_MARGIN)
COS_M = math.cos(M_MARGIN)
COS_TH = math.cos(math.pi - M_MARGIN)
MM = math.sin(math.pi - M_MARGIN) * M_MARGIN

LN_PRESCALE = 2.0**-40
LSE_SHIFT = 40.0 * math.log(2.0)


def build():
    nc = bacc.Bacc("TRN2", target_bir_lowering=False, debug=False, num_devices=NCORES)

    f32 = mybir.dt.float32
    x = nc.dram_tensor("logits", [R, C], f32, kind="ExternalInput").ap()
    lv = nc.dram_tensor("lv", [P, RB], f32, kind="ExternalInput").ap()
    out = nc.dram_tensor("out", [1, 1], f32, kind="ExternalOutput").ap()

    xt = x.rearrange("(rb p) c -> rb p c", p=P)

    Exp = mybir.ActivationFunctionType.Exp
    Ln = mybir.ActivationFunctionType.Ln
    Alu = mybir.AluOpType

    with tile.TileContext(nc) as tc:
        with (
            tc.tile_pool(name="small", bufs=1) as small,
            tc.tile_pool(name="inp", bufs=3) as inp,
            tc.tile_pool(name="scr", bufs=2) as scrp,
        ):
            lv_sb = small.tile([P, RB], f32)
            nc.sync.dma_start(out=lv_sb, in_=lv)

            # Label-column tail, part 1. Emitted first so the ACT table load
            # happens while the first bulk tile is still streaming in.
            lv2 = small.tile([P, RB], f32)
            nc.vector.tensor_mul(lv2, lv_sb, lv_sb)
            omlv2 = small.tile([P, RB], f32)  # 1 - lv^2
            nc.vector.tensor_scalar(
                out=omlv2, in0=lv2, scalar1=-1.0, scalar2=1.0, op0=Alu.mult, op1=Alu.add
            )
            lns = small.tile([P, RB], f32)
            nc.scalar.activation(lns, omlv2, Ln)
            sine = small.tile([P, RB], f32)  # sqrt(1-lv^2) = exp(0.5*ln(1-lv^2))
            nc.scalar.activation(sine, lns, Exp, scale=0.5)
            sms = small.tile([P, RB], f32)
            nc.vector.tensor_scalar_mul(sms, sine, SIN_M)
            phi = small.tile([P, RB], f32)  # cos(theta + m)
            nc.vector.scalar_tensor_tensor(
                out=phi, in0=lv_sb, scalar=COS_M, in1=sms, op0=Alu.mult, op1=Alu.subtract
            )
            # easy-margin fallback: where(lv > COS_TH, phi, lv - MM)
            mask = small.tile([P, RB], f32)
            nc.vector.tensor_scalar(
                out=mask, in0=lv_sb, scalar1=COS_TH, scalar2=None, op0=Alu.is_gt
            )
            alt = small.tile([P, RB], f32)
            nc.vector.tensor_scalar_sub(alt, lv_sb, MM)
            dphi = small.tile([P, RB], f32)
            nc.vector.tensor_sub(dphi, phi, alt)
            mdp = small.tile([P, RB], f32)
            nc.vector.tensor_mul(mdp, mask, dphi)
            phisel = small.tile([P, RB], f32)
            nc.vector.tensor_add(phisel, alt, mdp)
            sl = small.tile([P, RB], f32)  # S * phi  (the label logit)
            nc.vector.tensor_scalar_mul(sl, phisel, S)
            e_new = small.tile([P, RB], f32)
            nc.scalar.activation(e_new, phisel, Exp, scale=S)
            e_old = small.tile([P, RB], f32)
            nc.scalar.activation(e_old, lv_sb, Exp, scale=S)

            # Bulk: per-row sum of exp(S*x), fused into the ACT pass via
            # accum_out. One [P, F] tile per DMA; exp output goes to a
            # rotating scratch that is never read.
            acc = small.tile([P, RB, T], f32)
            for rb in range(RB):
                for t in range(T):
                    x_tile = inp.tile([P, F], f32)
                    nc.sync.dma_start(out=x_tile, in_=xt[rb, :, t * F : (t + 1) * F])
                    scr = scrp.tile([P, F], f32)
                    nc.scalar.activation(
                        scr, x_tile, Exp, scale=S, accum_out=acc[:, rb, t : t + 1]
                    )

            rowsum = small.tile([P, RB], f32)
            nc.vector.reduce_sum(rowsum, acc, axis=mybir.AxisListType.X)

            # Tail part 2: swap the label column's exp for the margin one,
            # then log-sum-exp and the per-row loss.
            t1 = small.tile([P, RB], f32)
            nc.vector.tensor_sub(t1, rowsum, e_old)
            adj = small.tile([P, RB], f32)
            nc.vector.tensor_add(adj, t1, e_new)
            lse = small.tile([P, RB], f32)
            nc.scalar.activation(lse, adj, Ln, scale=LN_PRESCALE)
            lossr = small.tile([P, RB], f32)  # lse + 40ln2 - S*phi
            nc.vector.scalar_tensor_tensor(
                out=lossr, in0=lse, scalar=LSE_SHIFT, in1=sl, op0=Alu.add, op1=Alu.subtract
            )
            red = small.tile([P, RB], f32)
            nc.gpsimd.partition_all_reduce(red, lossr, P, ReduceOp.add)
            tot = small.tile([1, 1], f32)
            nc.vector.tensor_add(tot, red[0:1, 0:1], red[0:1, 1:2])
            outv = small.tile([1, 1], f32)
            nc.vector.tensor_scalar_mul(outv, tot, 1.0 / N)
            nc.sync.dma_start(out=out, in_=outv)

    nc.compile()
    return nc


_NC_CACHE = None


def _get_nc():
    global _NC_CACHE
    if _NC_CACHE is None:
        _NC_CACHE = build()
    return _NC_CACHE


def make_in_maps(logits, labels):
    logits = np.ascontiguousarray(np.asarray(logits), dtype=np.float32)
    labels = np.asarray(labels).astype(np.int64).ravel()
    assert logits.shape == (N, C), logits.shape
    assert labels.shape == (N,), labels.shape
    lv_all = logits[np.arange(N), labels].astype(np.float32)
    in_maps = []
    for i in range(NCORES):
        shard = logits[i * R : (i + 1) * R]
        # lvi[p, rb] = label value of local row rb*128 + p
        lvi = np.ascontiguousarray(lv_all[i * R : (i + 1) * R].reshape(RB, P).T)
        in_maps.append({"logits": shard, "lv": lvi})
    return in_maps


def run(logits, labels, trace=False, trace_cores=None):
    in_maps = make_in_maps(logits, labels)
    nc = _get_nc()
    res = run_bass_kernel_spmd(
        nc,
        in_maps,
        core_ids=list(range(NCORES)),
        trace=trace,
        trace_cores=trace_cores,
    )
    total = 0.0
    for r in res.results:
        total += float(r["out"][0, 0])
    return np.float32(total), res


def kernel(logits, labels):
    loss, _ = run(logits, labels)
    return np.asarray(loss, dtype=np.float32)


# revision 6
# speedup vs baseline: 1.1083x; 1.1083x over previous
"""ArcFace loss (m=0.5, s=40) on 8 TRN2 NeuronCores.

Full inputs -> batch-sharded across 8 cores (rows), each core computes the
loss contribution of its 256 rows fully locally, host sums 8 partial scalars.

Per-row math (margin only touches the label column):
    rowsum = sum_c exp(S * x[c])
    adj    = rowsum - exp(S * x_lbl) + exp(S * phi(x_lbl))
    loss   = log(adj) - S * phi(x_lbl)
S * x < 40, so exp never overflows f32 and no max-subtraction pass is needed.
log(adj) is computed as Ln(adj * 2^-40) + 40*ln2 to stay inside the ScalarE
Ln table's valid input range (+-2^64).
sqrt(1-x^2) is computed as exp(0.5*ln(1-x^2)) so every ACT op lives in the
single natural_log_exp table set (one table load total).
"""

import math

import numpy as np

import concourse.bacc as bacc
import concourse.mybir as mybir
import concourse.tile as tile
from concourse.bass_isa import ReduceOp
from concourse.bass_utils import run_bass_kernel_spmd

# Problem shape (hardcoded per harness contract).
N, C = 2048, 32768
NCORES = 8
R = N // NCORES  # rows per core = 256
P = 128  # SBUF partitions
RB = R // P  # row blocks per core = 2

# Column tile widths per row block. The globally-last tiles taper down so
# the final exp (which cannot start until the last DMA lands) is short.
COL_TILES = [
    [8192, 8192, 8192, 8192],
    [8192, 8192, 8192, 4096, 2048, 1024, 1024],
]
assert all(sum(t) == C for t in COL_TILES)
FMAX = max(max(t) for t in COL_TILES)

# ArcFace constants (m=0.5, s=40).
M_MARGIN = 0.5
S = 40.0
SIN_M = math.sin(M_MARGIN)
COS_M = math.cos(M_MARGIN)
COS_TH = math.cos(math.pi - M_MARGIN)
MM = math.sin(math.pi - M_MARGIN) * M_MARGIN

LN_PRESCALE = 2.0**-40
LSE_SHIFT = 40.0 * math.log(2.0)


def _patched_act_tables(orig):
    """Restrict Exp/Ln to the combined natural_log_exp set so the table-load
    pass keeps one table resident for the whole kernel (the default greedy
    choice splits them across two sets, putting a ~1.3us table load on the
    critical tail right before the final Ln)."""

    def patched(arch):
        tabs = orig(arch)
        Exp = mybir.ActivationFunctionType.Exp
        Ln = mybir.ActivationFunctionType.Ln
        out = {}
        for name, funcs in tabs.items():
            if name != "natural_log_exp_and_others":
                funcs = funcs - {Exp, Ln}
            out[name] = funcs
        return out

    return patched


def build():
    nc = bacc.Bacc("TRN2", target_bir_lowering=False, debug=False, num_devices=NCORES)

    f32 = mybir.dt.float32
    x = nc.dram_tensor("logits", [R, C], f32, kind="ExternalInput").ap()
    lv = nc.dram_tensor("lv", [P, RB], f32, kind="ExternalInput").ap()
    out = nc.dram_tensor("out", [1, 1], f32, kind="ExternalOutput").ap()

    xt = x.rearrange("(rb p) c -> rb p c", p=P)

    Exp = mybir.ActivationFunctionType.Exp
    Ln = mybir.ActivationFunctionType.Ln
    Alu = mybir.AluOpType

    with tile.TileContext(nc) as tc:
        with (
            tc.tile_pool(name="small", bufs=1) as small,
            tc.tile_pool(name="inp", bufs=3) as inp,
            tc.tile_pool(name="scr", bufs=2) as scrp,
        ):
            lv_sb = small.tile([P, RB], f32)
            # SWDGE so the SP (HWDGE) ring's first descriptor is the first
            # bulk tile — the lv load must not delay the main stream.
            nc.gpsimd.dma_start(out=lv_sb, in_=lv)

            # Label-column tail, part 1. Emitted first so the ACT table load
            # happens while the first bulk tile is still streaming in.
            lv2 = small.tile([P, RB], f32)
            nc.vector.tensor_mul(lv2, lv_sb, lv_sb)
            omlv2 = small.tile([P, RB], f32)  # 1 - lv^2
            nc.vector.tensor_scalar(
                out=omlv2, in0=lv2, scalar1=-1.0, scalar2=1.0, op0=Alu.mult, op1=Alu.add
            )
            lns = small.tile([P, RB], f32)
            nc.scalar.activation(lns, omlv2, Ln)
            sine = small.tile([P, RB], f32)  # sqrt(1-lv^2) = exp(0.5*ln(1-lv^2))
            nc.scalar.activation(sine, lns, Exp, scale=0.5)
            sms = small.tile([P, RB], f32)
            nc.vector.tensor_scalar_mul(sms, sine, SIN_M)
            phi = small.tile([P, RB], f32)  # cos(theta + m)
            nc.vector.scalar_tensor_tensor(
                out=phi, in0=lv_sb, scalar=COS_M, in1=sms, op0=Alu.mult, op1=Alu.subtract
            )
            # easy-margin fallback: where(lv > COS_TH, phi, lv - MM)
            mask = small.tile([P, RB], f32)
            nc.vector.tensor_scalar(
                out=mask, in0=lv_sb, scalar1=COS_TH, scalar2=None, op0=Alu.is_gt
            )
            alt = small.tile([P, RB], f32)
            nc.vector.tensor_scalar_sub(alt, lv_sb, MM)
            dphi = small.tile([P, RB], f32)
            nc.vector.tensor_sub(dphi, phi, alt)
            mdp = small.tile([P, RB], f32)
            nc.vector.tensor_mul(mdp, mask, dphi)
            phisel = small.tile([P, RB], f32)
            nc.vector.tensor_add(phisel, alt, mdp)
            sl = small.tile([P, RB], f32)  # S * phi  (the label logit)
            nc.vector.tensor_scalar_mul(sl, phisel, S)
            e_new = small.tile([P, RB], f32)
            nc.scalar.activation(e_new, phisel, Exp, scale=S)
            e_old = small.tile([P, RB], f32)
            nc.scalar.activation(e_old, lv_sb, Exp, scale=S)

            # Bulk: per-row sum of exp(S*x), fused into the ACT pass via
            # accum_out. One [P, F] tile per DMA; exp output goes to a
            # rotating scratch that is never read.
            ntiles = sum(len(t) for t in COL_TILES)
            acc = small.tile([P, ntiles], f32)
            ti = 0
            rb_cols = []  # acc column range per row block
            for rb in range(RB):
                c0 = 0
                t0 = ti
                for w in COL_TILES[rb]:
                    x_tile = inp.tile([P, FMAX], f32, tag="x_tile")
                    nc.sync.dma_start(
                        out=x_tile[:, :w], in_=xt[rb, :, c0 : c0 + w]
                    )
                    scr = scrp.tile([P, FMAX], f32, tag="scr")
                    nc.scalar.activation(
                        scr[:, :w], x_tile[:, :w], Exp, scale=S,
                        accum_out=acc[:, ti : ti + 1],
                    )
                    c0 += w
                    ti += 1
                rb_cols.append((t0, ti))

            rowsum = small.tile([P, RB], f32)
            for rb, (a, b) in enumerate(rb_cols):
                nc.vector.reduce_sum(
                    rowsum[:, rb : rb + 1], acc[:, a:b], axis=mybir.AxisListType.X
                )

            # Tail part 2: swap the label column's exp for the margin one,
            # then log-sum-exp and the per-row loss.
            t1 = small.tile([P, RB], f32)
            nc.vector.tensor_sub(t1, rowsum, e_old)
            adj = small.tile([P, RB], f32)
            nc.vector.tensor_add(adj, t1, e_new)
            lse = small.tile([P, RB], f32)
            nc.scalar.activation(lse, adj, Ln, scale=LN_PRESCALE)
            lossr = small.tile([P, RB], f32)  # lse + 40ln2 - S*phi
            nc.vector.scalar_tensor_tensor(
                out=lossr, in0=lse, scalar=LSE_SHIFT, in1=sl, op0=Alu.add, op1=Alu.subtract
            )
            red = small.tile([P, RB], f32)
            nc.gpsimd.partition_all_reduce(red, lossr, P, ReduceOp.add)
            tot = small.tile([1, 1], f32)
            nc.vector.tensor_add(tot, red[0:1, 0:1], red[0:1, 1:2])
            outv = small.tile([1, 1], f32)
            nc.vector.tensor_scalar_mul(outv, tot, 1.0 / N)
            nc.sync.dma_start(out=out, in_=outv)

    orig_tables = bacc.get_activation_tables
    bacc.get_activation_tables = _patched_act_tables(orig_tables)
    try:
        nc.compile()
    finally:
        bacc.get_activation_tables = orig_tables
    return nc


_NC_CACHE = None


def _get_nc():
    global _NC_CACHE
    if _NC_CACHE is None:
        _NC_CACHE = build()
    return _NC_CACHE


def make_in_maps(logits, labels):
    logits = np.ascontiguousarray(np.asarray(logits), dtype=np.float32)
    labels = np.asarray(labels).astype(np.int64).ravel()
    assert logits.shape == (N, C), logits.shape
    assert labels.shape == (N,), labels.shape
    lv_all = logits[np.arange(N), labels].astype(np.float32)
    in_maps = []
    for i in range(NCORES):
        shard = logits[i * R : (i + 1) * R]
        # lvi[p, rb] = label value of local row rb*128 + p
        lvi = np.ascontiguousarray(lv_all[i * R : (i + 1) * R].reshape(RB, P).T)
        in_maps.append({"logits": shard, "lv": lvi})
    return in_maps


def run(logits, labels, trace=False, trace_cores=None):
    in_maps = make_in_maps(logits, labels)
    nc = _get_nc()
    res = run_bass_kernel_spmd(
        nc,
        in_maps,
        core_ids=list(range(NCORES)),
        trace=trace,
        trace_cores=trace_cores,
    )
    total = 0.0
    for r in res.results:
        total += float(r["out"][0, 0])
    return np.float32(total), res


def kernel(logits, labels):
    loss, _ = run(logits, labels)
    return np.asarray(loss, dtype=np.float32)


# revision 10
# speedup vs baseline: 1.1147x; 1.0057x over previous
"""ArcFace loss (m=0.5, s=40) on 8 TRN2 NeuronCores.

Full inputs -> batch-sharded across 8 cores (256 rows each, a contiguous
32 MB slab per core); each core computes the loss contribution of its rows
fully locally; the host sums the 8 partial scalars (the unshard step).

Per-row math (the ArcFace margin only touches the label column):
    rowsum = sum_c exp(S * x[c])
    adj    = rowsum - exp(S * x_lbl) + exp(S * phi(x_lbl))
    loss   = log(adj) - S * phi(x_lbl)
S * x < 40, so exp never overflows f32 and no max-subtraction pass is
needed -> single streaming pass over the data (memory-bound).
log(adj) is computed as Ln(adj * 2^-40) + 40*ln2 to stay inside the ScalarE
Ln table's valid input range (+-2^64). sqrt(1-x^2) is computed as
exp(0.5*ln(1-x^2)) so every ACT op lives in the single natural_log_exp
table set (one table load for the whole kernel).

Device kernel (raw bacc, hand-placed semaphores — no Tile entry/exit
barriers): SP streams 12 column tiles per core over the HWDGE ring
(~380 GB/s effective); ScalarE consumes each tile with one fused
exp(40x)+row-accumulate ACTIVATE; a tiny DVE/ACT/GPSIMD tail computes the
label-column fixup, log-sum-exp, and the partition sum. The final column
tiles taper down so the last exp after the stream ends is short.

Sync rules (HW-verified the hard way):
- Adjacent same-engine instructions overlap execution, so DEPENDENT
  same-engine pairs need inc@complete + wait just like cross-engine pairs.
- One DMA semaphore per buffer slot: a single sem shared by in-flight DMAs
  (+16 each) is racy because the 16 SDMA engine slices interleave.
"""

import math

import numpy as np

import concourse.bacc as bacc
import concourse.mybir as mybir
from concourse.bass_isa import ReduceOp
from concourse.bass_utils import run_bass_kernel_spmd

# Problem shape (hardcoded per harness contract).
N, C = 2048, 32768
NCORES = 8
R = N // NCORES  # rows per core = 256
P = 128  # SBUF partitions
RB = R // P  # row blocks per core = 2

# Column tile widths per row block. The globally-last tiles taper down so
# the final exp (which cannot start until the last DMA lands) is short.
COL_TILES = [
    [8192, 8192, 8192, 8192],
    [8192, 8192, 8192, 4096, 2048, 1024, 512, 512],
]
assert all(sum(t) == C for t in COL_TILES)
FMAX = max(max(t) for t in COL_TILES)
BUFS = 3

# ArcFace constants (m=0.5, s=40).
M_MARGIN = 0.5
S = 40.0
SIN_M = math.sin(M_MARGIN)
COS_M = math.cos(M_MARGIN)
COS_TH = math.cos(math.pi - M_MARGIN)
MM = math.sin(math.pi - M_MARGIN) * M_MARGIN

LN_PRESCALE = 2.0**-40
LSE_SHIFT = 40.0 * math.log(2.0)


def _patched_act_tables(orig):
    """Restrict Exp/Ln to the combined natural_log_exp set so the table-load
    pass keeps one table resident for the whole kernel (the default greedy
    choice splits them across two sets, putting a ~1.3us table load on the
    critical tail right before the final Ln)."""

    def patched(arch):
        tabs = orig(arch)
        Exp = mybir.ActivationFunctionType.Exp
        Ln = mybir.ActivationFunctionType.Ln
        out = {}
        for name, funcs in tabs.items():
            if name != "natural_log_exp_and_others":
                funcs = funcs - {Exp, Ln}
            out[name] = funcs
        return out

    return patched


def build():
    # detect_race_conditions=False: the checker does not model same-engine
    # program order; all cross/same-engine edges here carry explicit sems.
    nc = bacc.Bacc(
        "TRN2",
        target_bir_lowering=False,
        debug=False,
        num_devices=NCORES,
        detect_race_conditions=False,
    )

    f32 = mybir.dt.float32
    x = nc.dram_tensor("logits", [R, C], f32, kind="ExternalInput").ap()
    lv = nc.dram_tensor("lv", [P, RB], f32, kind="ExternalInput").ap()
    out = nc.dram_tensor("out", [1, 1], f32, kind="ExternalOutput").ap()

    xt = x.rearrange("(rb p) c -> rb p c", p=P)

    Exp = mybir.ActivationFunctionType.Exp
    Ln = mybir.ActivationFunctionType.Ln
    Alu = mybir.AluOpType

    tiles = []  # (rb, c0, width)
    for rb in range(RB):
        c0 = 0
        for w in COL_TILES[rb]:
            tiles.append((rb, c0, w))
            c0 += w
    ntiles = len(tiles)
    rb_cols = []
    i0 = 0
    for rb in range(RB):
        rb_cols.append((i0, i0 + len(COL_TILES[rb])))
        i0 += len(COL_TILES[rb])

    def sb(name, shape):
        return nc.alloc_sbuf_tensor(name, list(shape), f32).ap()

    bufs = [sb(f"buf{i}", [P, FMAX]) for i in range(BUFS)]
    scrs = [sb(f"scr{i}", [P, FMAX]) for i in range(2)]
    lv_sb = sb("lv_sb", [P, RB])
    lv2 = sb("lv2", [P, RB])
    omlv2 = sb("omlv2", [P, RB])
    lns = sb("lns", [P, RB])
    sine = sb("sine", [P, RB])
    sms = sb("sms", [P, RB])
    phi = sb("phi", [P, RB])
    mask = sb("mask", [P, RB])
    alt = sb("alt", [P, RB])
    dphi = sb("dphi", [P, RB])
    mdp = sb("mdp", [P, RB])
    phisel = sb("phisel", [P, RB])
    sl = sb("sl", [P, RB])
    e_new = sb("e_new", [P, RB])
    e_old = sb("e_old", [P, RB])
    ediff = sb("ediff", [P, RB])
    acc = sb("acc", [P, ntiles])
    rowsum = sb("rowsum", [P, RB])
    adj = sb("adj", [P, RB])
    lse = sb("lse", [P, RB])
    lossr = sb("lossr", [P, RB])
    red = sb("red", [P, RB])
    sdummy = sb("sdummy", [1, RB])
    outv = sb("outv", [1, 1])

    s_in = [nc.alloc_semaphore(f"s_in{i}") for i in range(BUFS)]
    s_out = nc.alloc_semaphore("s_out")
    s_lv = nc.alloc_semaphore("s_lv")
    s_a = nc.alloc_semaphore("s_a")  # ACT milestones, +1
    s_v = nc.alloc_semaphore("s_v")  # DVE milestones, +1
    s_g = nc.alloc_semaphore("s_g")  # gpsimd milestones, +1
    all_sems = [*s_in, s_out, s_lv, s_a, s_v, s_g]

    va = 0
    vv = 0

    def act(ins):
        nonlocal va
        va += 1
        ins.then_inc(s_a, 1)
        return va

    def dve(ins):
        nonlocal vv
        vv += 1
        ins.then_inc(s_v, 1)
        return vv

    # ---- gpsimd: lv load (SWDGE keeps the SP ring free for the stream)
    nc.gpsimd.dma_start(out=lv_sb, in_=lv).then_inc(s_lv, 16)

    # ---- DVE: label-column prep (needs lv)
    nc.vector.wait_ge(s_lv, 16)
    v_lv2 = dve(nc.vector.tensor_mul(lv2, lv_sb, lv_sb))
    v_mask = dve(
        nc.vector.tensor_scalar(
            out=mask, in0=lv_sb, scalar1=COS_TH, scalar2=None, op0=Alu.is_gt
        )
    )
    v_alt = dve(nc.vector.tensor_scalar_sub(alt, lv_sb, MM))
    nc.vector.wait_ge(s_v, v_lv2)
    v_omlv2 = dve(
        nc.vector.tensor_scalar(
            out=omlv2, in0=lv2, scalar1=-1.0, scalar2=1.0, op0=Alu.mult, op1=Alu.add
        )
    )

    # ---- ACT: sqrt(1-lv^2) via exp(0.5*ln(.)), e_old
    nc.scalar.wait_ge(s_v, v_omlv2)
    a_lns = act(nc.scalar.activation(lns, omlv2, Ln))
    nc.scalar.wait_ge(s_a, a_lns)
    a_sine = act(nc.scalar.activation(sine, lns, Exp, scale=0.5))
    act(nc.scalar.activation(e_old, lv_sb, Exp, scale=S))

    # ---- DVE: phi chain (margin-adjusted label logit), each link semmed
    nc.vector.wait_ge(s_a, a_sine)
    v_sms = dve(nc.vector.tensor_scalar_mul(sms, sine, SIN_M))
    nc.vector.wait_ge(s_v, v_sms)
    v_phi = dve(
        nc.vector.scalar_tensor_tensor(
            out=phi, in0=lv_sb, scalar=COS_M, in1=sms, op0=Alu.mult, op1=Alu.subtract
        )
    )
    nc.vector.wait_ge(s_v, max(v_phi, v_alt))
    v_dphi = dve(nc.vector.tensor_sub(dphi, phi, alt))
    nc.vector.wait_ge(s_v, max(v_dphi, v_mask))
    v_mdp = dve(nc.vector.tensor_mul(mdp, mask, dphi))
    nc.vector.wait_ge(s_v, v_mdp)
    v_phisel = dve(nc.vector.tensor_add(phisel, alt, mdp))
    nc.vector.wait_ge(s_v, v_phisel)
    v_sl = dve(nc.vector.tensor_scalar_mul(sl, phisel, S))

    # ---- ACT: e_new (needs phisel)
    nc.scalar.wait_ge(s_v, v_phisel)
    a_enew = act(nc.scalar.activation(e_new, phisel, Exp, scale=S))

    # ---- DVE: ediff (e_old's write precedes e_new's inc on the same engine)
    nc.vector.wait_ge(s_a, a_enew)
    v_ediff = dve(nc.vector.tensor_sub(ediff, e_new, e_old))

    # ---- SP: bulk input stream; slot k%BUFS recycled once ACT consumed
    # tile k-BUFS (ACT milestone for bulk tile j is a_enew+1+j).
    for k in range(ntiles):
        rb, c0, w = tiles[k]
        if k >= BUFS:
            nc.sync.wait_ge(s_a, a_enew + 1 + (k - BUFS))
        nc.sync.dma_start(
            out=bufs[k % BUFS][:, :w], in_=xt[rb, :, c0 : c0 + w]
        ).then_inc(s_in[k % BUFS], 16)

    # ---- ACT: bulk exp + fused row-sum (accum_out); exp data output goes
    # to a rotating scratch that is never read.
    a_tiles = []
    for j in range(ntiles):
        rb, c0, w = tiles[j]
        nc.scalar.wait_ge(s_in[j % BUFS], 16 * (j // BUFS + 1))
        a_tiles.append(
            act(
                nc.scalar.activation(
                    scrs[j % 2][:, :w],
                    bufs[j % BUFS][:, :w],
                    Exp,
                    scale=S,
                    accum_out=acc[:, j : j + 1],
                )
            )
        )

    # ---- DVE: row sums -> adjusted logsumexp input
    nc.vector.wait_ge(s_a, a_tiles[-1])
    v_rs = []
    for rb, (a, b) in enumerate(rb_cols):
        v_rs.append(
            dve(
                nc.vector.reduce_sum(
                    rowsum[:, rb : rb + 1], acc[:, a:b], axis=mybir.AxisListType.X
                )
            )
        )
    nc.vector.wait_ge(s_v, max(*v_rs, v_ediff))
    v_adj = dve(nc.vector.tensor_add(adj, rowsum, ediff))

    # ---- ACT: lse = Ln(adj * 2^-40)
    nc.scalar.wait_ge(s_v, v_adj)
    a_lse = act(nc.scalar.activation(lse, adj, Ln, scale=LN_PRESCALE))

    # ---- DVE: per-row loss = lse + 40ln2 - S*phi
    nc.vector.wait_ge(s_a, a_lse)
    v_lossr = dve(
        nc.vector.scalar_tensor_tensor(
            out=lossr, in0=lse, scalar=LSE_SHIFT, in1=sl, op0=Alu.add, op1=Alu.subtract
        )
    )

    # ---- gpsimd: sum across partitions
    nc.gpsimd.wait_ge(s_v, v_lossr)
    nc.gpsimd.partition_all_reduce(red, lossr, P, ReduceOp.add).then_inc(s_g, 1)

    # ---- DVE: partial = (red[0,0] + red[0,1]) / N, fused via accum_out
    nc.vector.wait_ge(s_g, 1)
    v_out = dve(
        nc.vector.tensor_scalar(
            out=sdummy,
            in0=red[0:1, 0:RB],
            scalar1=1.0 / N,
            scalar2=0.0,
            op0=Alu.mult,
            op1=Alu.add,
            accum_out=outv,
        )
    )

    # ---- SP: result out, wait for landing
    nc.sync.wait_ge(s_v, v_out)
    nc.sync.dma_start(out=out, in_=outv).then_inc(s_out, 16)
    nc.sync.wait_ge(s_out, 16)

    # ---- epilogue: leave semaphores as the next execution expects them
    nc.all_engine_barrier()
    nums = [s.num for s in all_sems]
    nc.gpsimd.dma_reset(range(min(nums), max(nums) + 1))
    nc.gpsimd.sem_clear(range(min(nums), max(nums) + 1))

    orig_tables = bacc.get_activation_tables
    bacc.get_activation_tables = _patched_act_tables(orig_tables)
    try:
        nc.compile()
    finally:
        bacc.get_activation_tables = orig_tables
    return nc


_NC_CACHE = None


def _get_nc():
    global _NC_CACHE
    if _NC_CACHE is None:
        _NC_CACHE = build()
    return _NC_CACHE


def make_in_maps(logits, labels):
    logits = np.ascontiguousarray(np.asarray(logits), dtype=np.float32)
    labels = np.asarray(labels).astype(np.int64).ravel()
    assert logits.shape == (N, C), logits.shape
    assert labels.shape == (N,), labels.shape
    lv_all = logits[np.arange(N), labels].astype(np.float32)
    in_maps = []
    for i in range(NCORES):
        shard = logits[i * R : (i + 1) * R]
        # lvi[p, rb] = label-column value of local row rb*128 + p
        lvi = np.ascontiguousarray(lv_all[i * R : (i + 1) * R].reshape(RB, P).T)
        in_maps.append({"logits": shard, "lv": lvi})
    return in_maps


def run(logits, labels, trace=False, trace_cores=None):
    in_maps = make_in_maps(logits, labels)
    nc = _get_nc()
    res = run_bass_kernel_spmd(
        nc,
        in_maps,
        core_ids=list(range(NCORES)),
        trace=trace,
        trace_cores=trace_cores,
    )
    total = 0.0
    for r in res.results:
        total += float(r["out"][0, 0])
    return np.float32(total), res


def kernel(logits, labels):
    loss, _ = run(logits, labels)
    return np.asarray(loss, dtype=np.float32)


# revision 12
# speedup vs baseline: 1.1691x; 1.0489x over previous
"""ArcFace loss (m=0.5, s=40) on 8 TRN2 NeuronCores.

Full inputs -> batch-sharded across 8 cores (256 rows each, a contiguous
32 MB slab per core); each core computes the loss contribution of its rows
fully locally; the host sums the 8 partial scalars (the unshard step).

Per-row math (the ArcFace margin only touches the label column):
    rowsum = sum_c exp(S * x[c])
    adj    = rowsum - exp(S * x_lbl) + exp(S * phi(x_lbl))
    loss   = log(adj) - S * phi(x_lbl)
S * x < 40, so exp never overflows f32 and no max-subtraction pass is
needed -> single streaming pass over the data (memory-bound).
log(adj) is computed as Ln(adj * 2^-40) + 40*ln2 to stay inside the ScalarE
Ln table's valid input range (+-2^64). sqrt(1-x^2) is computed as
exp(0.5*ln(1-x^2)) so every ACT op lives in the single natural_log_exp
table set (one table load for the whole kernel).

Device kernel (raw bacc, hand-placed semaphores — no Tile entry/exit
barriers): SP streams 12 column tiles per core over the HWDGE ring
(~380 GB/s effective); ScalarE consumes each tile with one fused
exp(40x)+row-accumulate ACTIVATE; a tiny DVE/ACT/GPSIMD tail computes the
label-column fixup, log-sum-exp, and the partition sum. The final column
tiles taper down so the last exp after the stream ends is short.

Sync rules (HW-verified the hard way):
- Adjacent same-engine instructions overlap execution, so DEPENDENT
  same-engine pairs need inc@complete + wait just like cross-engine pairs.
- One DMA semaphore per buffer slot: a single sem shared by in-flight DMAs
  (+16 each) is racy because the 16 SDMA engine slices interleave.
"""

import math

import numpy as np

import concourse.bacc as bacc
import concourse.mybir as mybir
from concourse.bass_isa import ReduceOp
from concourse.bass_utils import run_bass_kernel_spmd

# Problem shape (hardcoded per harness contract).
N, C = 2048, 32768
NCORES = 8
R = N // NCORES  # rows per core = 256
P = 128  # SBUF partitions
RB = R // P  # row blocks per core = 2

# Column tile widths per row block. The globally-last tiles taper down so
# the final exp (which cannot start until the last DMA lands) is short.
COL_TILES = [
    [8192, 8192, 8192, 8192],
    [8192, 8192, 8192, 4096, 2048, 1024, 512, 256, 256],
]
assert all(sum(t) == C for t in COL_TILES)
FMAX = max(max(t) for t in COL_TILES)
BUFS = 3

# ArcFace constants (m=0.5, s=40).
M_MARGIN = 0.5
S = 40.0
SIN_M = math.sin(M_MARGIN)
COS_M = math.cos(M_MARGIN)
COS_TH = math.cos(math.pi - M_MARGIN)
MM = math.sin(math.pi - M_MARGIN) * M_MARGIN

LN_PRESCALE = 2.0**-40
LSE_SHIFT = 40.0 * math.log(2.0)


def _patched_act_tables(orig):
    """Restrict Exp/Ln to the combined natural_log_exp set so the table-load
    pass keeps one table resident for the whole kernel (the default greedy
    choice splits them across two sets, putting a ~1.3us table load on the
    critical tail right before the final Ln)."""

    def patched(arch):
        tabs = orig(arch)
        Exp = mybir.ActivationFunctionType.Exp
        Ln = mybir.ActivationFunctionType.Ln
        out = {}
        for name, funcs in tabs.items():
            if name != "natural_log_exp_and_others":
                funcs = funcs - {Exp, Ln}
            out[name] = funcs
        return out

    return patched


def build():
    # detect_race_conditions=False: the checker does not model same-engine
    # program order; all cross/same-engine edges here carry explicit sems.
    nc = bacc.Bacc(
        "TRN2",
        target_bir_lowering=False,
        debug=False,
        num_devices=NCORES,
        detect_race_conditions=False,
    )

    f32 = mybir.dt.float32
    x = nc.dram_tensor("logits", [R, C], f32, kind="ExternalInput").ap()
    lv = nc.dram_tensor("lv", [P, RB], f32, kind="ExternalInput").ap()
    out = nc.dram_tensor("out", [1, 1], f32, kind="ExternalOutput").ap()

    xt = x.rearrange("(rb p) c -> rb p c", p=P)

    Exp = mybir.ActivationFunctionType.Exp
    Ln = mybir.ActivationFunctionType.Ln
    Alu = mybir.AluOpType

    tiles = []  # (rb, c0, width)
    for rb in range(RB):
        c0 = 0
        for w in COL_TILES[rb]:
            tiles.append((rb, c0, w))
            c0 += w
    ntiles = len(tiles)
    rb_cols = []
    i0 = 0
    for rb in range(RB):
        rb_cols.append((i0, i0 + len(COL_TILES[rb])))
        i0 += len(COL_TILES[rb])

    def sb(name, shape):
        return nc.alloc_sbuf_tensor(name, list(shape), f32).ap()

    bufs = [sb(f"buf{i}", [P, FMAX]) for i in range(BUFS)]
    scrs = [sb(f"scr{i}", [P, FMAX]) for i in range(2)]
    lv_sb = sb("lv_sb", [P, RB])
    lv2 = sb("lv2", [P, RB])
    omlv2 = sb("omlv2", [P, RB])
    lns = sb("lns", [P, RB])
    sine = sb("sine", [P, RB])
    sms = sb("sms", [P, RB])
    phi = sb("phi", [P, RB])
    mask = sb("mask", [P, RB])
    alt = sb("alt", [P, RB])
    dphi = sb("dphi", [P, RB])
    mdp = sb("mdp", [P, RB])
    phisel = sb("phisel", [P, RB])
    sl = sb("sl", [P, RB])
    e_new = sb("e_new", [P, RB])
    e_old = sb("e_old", [P, RB])
    ediff = sb("ediff", [P, RB])
    acc = sb("acc", [P, ntiles])
    rowsum = sb("rowsum", [P, RB])
    adj = sb("adj", [P, RB])
    lse = sb("lse", [P, RB])
    lossr = sb("lossr", [P, RB])
    red = sb("red", [P, RB])
    sdummy = sb("sdummy", [1, RB])
    outv = sb("outv", [1, 1])

    s_in = [nc.alloc_semaphore(f"s_in{i}") for i in range(BUFS)]
    s_out = nc.alloc_semaphore("s_out")
    s_lv = nc.alloc_semaphore("s_lv")
    s_a = nc.alloc_semaphore("s_a")  # ACT milestones, +1
    s_v = nc.alloc_semaphore("s_v")  # DVE milestones, +1
    s_g = nc.alloc_semaphore("s_g")  # gpsimd milestones, +1
    all_sems = [*s_in, s_out, s_lv, s_a, s_v, s_g]

    va = 0
    vv = 0

    def act(ins):
        nonlocal va
        va += 1
        ins.then_inc(s_a, 1)
        return va

    def dve(ins):
        nonlocal vv
        vv += 1
        ins.then_inc(s_v, 1)
        return vv

    # ---- gpsimd: lv load (SWDGE keeps the SP ring free for the stream)
    nc.gpsimd.dma_start(out=lv_sb, in_=lv).then_inc(s_lv, 16)

    # ---- DVE: label-column prep (needs lv)
    nc.vector.wait_ge(s_lv, 16)
    v_lv2 = dve(nc.vector.tensor_mul(lv2, lv_sb, lv_sb))
    v_mask = dve(
        nc.vector.tensor_scalar(
            out=mask, in0=lv_sb, scalar1=COS_TH, scalar2=None, op0=Alu.is_gt
        )
    )
    v_alt = dve(nc.vector.tensor_scalar_sub(alt, lv_sb, MM))
    nc.vector.wait_ge(s_v, v_lv2)
    v_omlv2 = dve(
        nc.vector.tensor_scalar(
            out=omlv2, in0=lv2, scalar1=-1.0, scalar2=1.0, op0=Alu.mult, op1=Alu.add
        )
    )

    # ---- ACT: sqrt(1-lv^2) via exp(0.5*ln(.)), e_old
    nc.scalar.wait_ge(s_v, v_omlv2)
    a_lns = act(nc.scalar.activation(lns, omlv2, Ln))
    nc.scalar.wait_ge(s_a, a_lns)
    a_sine = act(nc.scalar.activation(sine, lns, Exp, scale=0.5))
    act(nc.scalar.activation(e_old, lv_sb, Exp, scale=S))

    # ---- DVE: phi chain (margin-adjusted label logit), each link semmed
    nc.vector.wait_ge(s_a, a_sine)
    v_sms = dve(nc.vector.tensor_scalar_mul(sms, sine, SIN_M))
    nc.vector.wait_ge(s_v, v_sms)
    v_phi = dve(
        nc.vector.scalar_tensor_tensor(
            out=phi, in0=lv_sb, scalar=COS_M, in1=sms, op0=Alu.mult, op1=Alu.subtract
        )
    )
    nc.vector.wait_ge(s_v, max(v_phi, v_alt))
    v_dphi = dve(nc.vector.tensor_sub(dphi, phi, alt))
    nc.vector.wait_ge(s_v, max(v_dphi, v_mask))
    v_mdp = dve(nc.vector.tensor_mul(mdp, mask, dphi))
    nc.vector.wait_ge(s_v, v_mdp)
    v_phisel = dve(nc.vector.tensor_add(phisel, alt, mdp))
    nc.vector.wait_ge(s_v, v_phisel)
    v_sl = dve(nc.vector.tensor_scalar_mul(sl, phisel, S))

    # ---- ACT: e_new (needs phisel)
    nc.scalar.wait_ge(s_v, v_phisel)
    a_enew = act(nc.scalar.activation(e_new, phisel, Exp, scale=S))

    # ---- DVE: ediff (e_old's write precedes e_new's inc on the same engine)
    nc.vector.wait_ge(s_a, a_enew)
    v_ediff = dve(nc.vector.tensor_sub(ediff, e_new, e_old))

    # ---- SP: bulk input stream; slot k%BUFS recycled once ACT consumed
    # tile k-BUFS (ACT milestone for bulk tile j is a_enew+1+j).
    for k in range(ntiles):
        rb, c0, w = tiles[k]
        if k >= BUFS:
            nc.sync.wait_ge(s_a, a_enew + 1 + (k - BUFS))
        nc.sync.dma_start(
            out=bufs[k % BUFS][:, :w], in_=xt[rb, :, c0 : c0 + w]
        ).then_inc(s_in[k % BUFS], 16)

    # ---- ACT: bulk exp + fused row-sum (accum_out); exp data output goes
    # to a rotating scratch that is never read.
    a_tiles = []
    for j in range(ntiles):
        rb, c0, w = tiles[j]
        nc.scalar.wait_ge(s_in[j % BUFS], 16 * (j // BUFS + 1))
        a_tiles.append(
            act(
                nc.scalar.activation(
                    scrs[j % 2][:, :w],
                    bufs[j % BUFS][:, :w],
                    Exp,
                    scale=S,
                    accum_out=acc[:, j : j + 1],
                )
            )
        )

    # ---- DVE: row sums -> adjusted logsumexp input
    nc.vector.wait_ge(s_a, a_tiles[-1])
    v_rs = []
    for rb, (a, b) in enumerate(rb_cols):
        v_rs.append(
            dve(
                nc.vector.reduce_sum(
                    rowsum[:, rb : rb + 1], acc[:, a:b], axis=mybir.AxisListType.X
                )
            )
        )
    nc.vector.wait_ge(s_v, max(*v_rs, v_ediff))
    v_adj = dve(nc.vector.tensor_add(adj, rowsum, ediff))

    # ---- ACT: lse = Ln(adj * 2^-40)
    nc.scalar.wait_ge(s_v, v_adj)
    a_lse = act(nc.scalar.activation(lse, adj, Ln, scale=LN_PRESCALE))

    # ---- DVE: per-row loss = lse + 40ln2 - S*phi
    nc.vector.wait_ge(s_a, a_lse)
    v_lossr = dve(
        nc.vector.scalar_tensor_tensor(
            out=lossr, in0=lse, scalar=LSE_SHIFT, in1=sl, op0=Alu.add, op1=Alu.subtract
        )
    )

    # ---- gpsimd: sum across partitions
    nc.gpsimd.wait_ge(s_v, v_lossr)
    nc.gpsimd.partition_all_reduce(red, lossr, P, ReduceOp.add).then_inc(s_g, 1)

    # ---- DVE: partial = (red[0,0] + red[0,1]) / N, fused via accum_out
    nc.vector.wait_ge(s_g, 1)
    v_out = dve(
        nc.vector.tensor_scalar(
            out=sdummy,
            in0=red[0:1, 0:RB],
            scalar1=1.0 / N,
            scalar2=0.0,
            op0=Alu.mult,
            op1=Alu.add,
            accum_out=outv,
        )
    )

    # ---- SP: result out, wait for landing
    nc.sync.wait_ge(s_v, v_out)
    nc.sync.dma_start(out=out, in_=outv).then_inc(s_out, 16)
    nc.sync.wait_ge(s_out, 16)

    # ---- epilogue: leave semaphores as the next execution expects them.
    # No dma_reset: every DMA has been completion-waited via its semaphore,
    # so there is no in-flight DGE state, and the gpsimd dge_drain it lowers
    # to costs microseconds inside the measured window.
    nc.all_engine_barrier()
    nums = [s.num for s in all_sems]
    nc.gpsimd.sem_clear(range(min(nums), max(nums) + 1))

    orig_tables = bacc.get_activation_tables
    bacc.get_activation_tables = _patched_act_tables(orig_tables)
    try:
        nc.compile()
    finally:
        bacc.get_activation_tables = orig_tables
    return nc


_NC_CACHE = None


def _get_nc():
    global _NC_CACHE
    if _NC_CACHE is None:
        _NC_CACHE = build()
    return _NC_CACHE


def make_in_maps(logits, labels):
    logits = np.ascontiguousarray(np.asarray(logits), dtype=np.float32)
    labels = np.asarray(labels).astype(np.int64).ravel()
    assert logits.shape == (N, C), logits.shape
    assert labels.shape == (N,), labels.shape
    lv_all = logits[np.arange(N), labels].astype(np.float32)
    in_maps = []
    for i in range(NCORES):
        shard = logits[i * R : (i + 1) * R]
        # lvi[p, rb] = label-column value of local row rb*128 + p
        lvi = np.ascontiguousarray(lv_all[i * R : (i + 1) * R].reshape(RB, P).T)
        in_maps.append({"logits": shard, "lv": lvi})
    return in_maps


def run(logits, labels, trace=False, trace_cores=None):
    in_maps = make_in_maps(logits, labels)
    nc = _get_nc()
    res = run_bass_kernel_spmd(
        nc,
        in_maps,
        core_ids=list(range(NCORES)),
        trace=trace,
        trace_cores=trace_cores,
    )
    total = 0.0
    for r in res.results:
        total += float(r["out"][0, 0])
    return np.float32(total), res


def kernel(logits, labels):
    loss, _ = run(logits, labels)
    return np.asarray(loss, dtype=np.float32)
